# revision 22
# baseline (speedup 1.0000x reference)
"""Additive-attention kernel for 8 TRN2 NeuronCores.

reference:
    x = concat([s, h], axis=1)            # (N, 2D)
    X = tanh(x @ W.T)                     # (N, 2*DA)
    pre = (X @ v.T).T                     # (1, N)
    out = softmax(pre, axis=1)            # (1, N)

Strategy: shard rows (N) across 8 cores (4096 rows each). W, v replicated.
Each core computes tanh(x_shard @ W.T) @ v.T fused in SBUF/PSUM (bf16
matmul, fp32 accumulate), produces 4096 scores, takes exp, sums locally,
AllGathers the 8 partial sums, and normalizes its shard by the global sum.
Softmax max-subtraction is skipped: |score| <= ||v||_1 ~ 33 << 88 (fp32 exp
overflow), so exp is always finite and the result is exact to fp32.

Host-side prep is layout only (transpose/concat/cast + replicate v).
"""

import numpy as np
import ml_dtypes

N, D, DA = 32768, 1024, 1024
NCORES = 8
NS = N // NCORES            # 4096 rows per core
P = 128
MT = NS // P                # 32 row-tiles per core
KIN = 2 * D                 # 2048 contraction
KT = KIN // P               # 16 k-tiles
NOUT = 2 * DA               # 2048 out features
NCH = 512                   # psum chunk (one bank of fp32)
NCK = NOUT // NCH           # 4 chunks


def _build_nc():
    from concourse import bacc, mybir, tile, bass_isa, bass

    f32 = mybir.dt.float32
    bf16 = mybir.dt.bfloat16
    AF = mybir.ActivationFunctionType
    ALU = mybir.AluOpType
    AX = mybir.AxisListType

    nc = bacc.Bacc(
        "TRN2",
        target_bir_lowering=False,
        debug=False,
        num_devices=NCORES,
    )

    xh = nc.declare_dram_parameter("xh", [NS, KIN], bf16, isOutput=False)
    wt = nc.declare_dram_parameter("wt", [KIN, NOUT], bf16, isOutput=False)
    vr = nc.declare_dram_parameter("vr", [P, NOUT], f32, isOutput=False)
    out_ext = nc.declare_dram_parameter("out", [P, MT], f32, isOutput=True)

    with tile.TileContext(nc) as tc:
        with (
            tc.tile_pool(name="wpool", bufs=1) as wpool,
            tc.tile_pool(name="xpool", bufs=3) as xpool,
            tc.tile_pool(name="tpool", bufs=2) as tpool,
            tc.tile_pool(name="spool", bufs=1) as spool,
            tc.tile_pool(name="ppool", bufs=2, space="PSUM") as ppool,
            tc.tile_pool(name="dpool", bufs=1, space="DRAM") as dpool,
        ):
            def load_xm(m, eng=None):
                t = xpool.tile([P, KIN], bf16, name="xm", tag="xm")
                (eng or nc.sync).dma_start(
                    out=t[:, :], in_=xh[m * P:(m + 1) * P, :]
                )
                return t

            # first row-tiles go on the DMA queue BEFORE the 8.4MB of
            # weights so the PE can start within a few us; spread the first
            # issues across engine queues (each dma_start costs ~0.6us of
            # descriptor generation on its issuing engine)
            xm_pre = [load_xm(0, nc.sync), load_xm(1, nc.gpsimd)]

            # rendezvous the 8 cores while the weight DMAs stream in, so the
            # real AllGather at the softmax doesn't pay launch-skew latency
            sync_in = dpool.tile([1, 1], f32, name="sync_in")
            sync_out = dpool.tile(
                [1, NCORES], f32, name="sync_out", addr_space="Shared"
            )
            nc.gpsimd.collective_compute(
                "AllGather",
                ALU.bypass,
                replica_groups=[list(range(NCORES))],
                ins=[sync_in.opt()],
                outs=[sync_out.opt()],
            )

            # wk0 in two halves on their own engine queues so the first
            # matmuls can start early; remaining weight tiles stream behind
            wsb = []
            for k in range(KT):
                wk = wpool.tile([P, NOUT], bf16, name=f"wk{k}")
                if k == 0:
                    half = NOUT // 2
                    nc.scalar.dma_start(
                        out=wk[:, 0:half], in_=wt[0:P, 0:half]
                    )
                    nc.sync.dma_start(
                        out=wk[:, half:NOUT], in_=wt[0:P, half:NOUT]
                    )
                else:
                    nc.sync.dma_start(
                        out=wk[:, :], in_=wt[k * P:(k + 1) * P, :]
                    )
                wsb.append(wk)
            vsb = wpool.tile([P, NOUT], f32, name="vsb")
            nc.sync.dma_start(out=vsb[:, :], in_=vr[:, :])

            scores = spool.tile([P, MT], f32, name="scores")
            expv = spool.tile([P, MT], f32, name="expv")
            zrow = spool.tile([P, 1], f32, name="zrow")
            ecol = spool.tile([P, 1], f32, name="ecol")

            for m in range(MT):
                xm = xm_pre[m] if m < len(xm_pre) else load_xm(m)


                psums = []
                for j in range(NCK):
                    ps = ppool.tile([P, NCH], f32, name=f"ps{j}", tag=f"ps{j}")
                    psums.append(ps)
                tmt = tpool.tile([P, NOUT], f32, name="tmt", tag="tmt")
                umt = tpool.tile([P, NOUT], f32, name="umt", tag="umt")
                acc = tpool.tile([P, NCK], f32, name="acc", tag="acc")

                def reduce_chunk(j):
                    sl = slice(j * NCH, (j + 1) * NCH)
                    nc.scalar.activation(tmt[:, sl], psums[j][:, :], AF.Tanh)
                    # one DVE op: umt = tanh*v, acc[:,j] = row-sum(umt)
                    nc.vector.scalar_tensor_tensor(
                        out=umt[:, sl],
                        in0=tmt[:, sl],
                        scalar=1.0,
                        in1=vsb[:, sl],
                        op0=ALU.mult,
                        op1=ALU.mult,
                        accum_out=acc[:, j:j + 1],
                    )

                if m < MT - 1:
                    # k-outer: one stationary load per (m, k), 4 matmuls
                    for k in range(KT):
                        lhs = xm[:, k * P:(k + 1) * P]
                        for j in range(NCK):
                            nc.tensor.matmul(
                                psums[j][:, :],
                                lhsT=lhs,
                                rhs=wsb[k][:, j * NCH:(j + 1) * NCH],
                                start=(k == 0),
                                stop=(k == KT - 1),
                            )
                    for j in range(NCK):
                        reduce_chunk(j)
                else:
                    # last tile: chunk-major so chunk j's tanh+reduce overlap
                    # chunk j+1's matmuls; only the final chunk's chain is
                    # exposed after the last matmul
                    for j in range(NCK):
                        for k in range(KT):
                            nc.tensor.matmul(
                                psums[j][:, :],
                                lhsT=xm[:, k * P:(k + 1) * P],
                                rhs=wsb[k][:, j * NCH:(j + 1) * NCH],
                                start=(k == 0),
                                stop=(k == KT - 1),
                            )
                        reduce_chunk(j)
                nc.vector.tensor_reduce(
                    scores[:, m:m + 1], acc[:, :], AX.X, ALU.add
                )
                # exp + z accumulation incrementally, hidden under the next
                # tile's matmuls; only the last column's exp is on the
                # critical path to the collective
                nc.scalar.activation(
                    expv[:, m:m + 1], scores[:, m:m + 1], AF.Exp,
                    accum_out=ecol[:, 0:1],
                )
                if m == 0:
                    nc.vector.tensor_copy(out=zrow[:, 0:1], in_=ecol[:, 0:1])
                else:
                    nc.vector.tensor_tensor(
                        zrow[:, 0:1], zrow[:, 0:1], ecol[:, 0:1], ALU.add
                    )

            # ---- softmax over the global N via one AllGather ----
            zloc = spool.tile([1, 1], f32, name="zloc")
            nc.gpsimd.tensor_reduce(
                zloc[0:1, 0:1], zrow[:, 0:1], AX.C, ALU.add
            )
            zin = dpool.tile([1, 1], f32, name="zin")
            zout = dpool.tile([1, NCORES], f32, name="zout", addr_space="Shared")
            nc.gpsimd.dma_start(out=zin[:, :], in_=zloc[0:1, 0:1])
            nc.gpsimd.collective_compute(
                "AllGather",
                ALU.bypass,
                replica_groups=[list(range(NCORES))],
                ins=[zin.opt()],
                outs=[zout.opt()],
            )
            # DMA the gathered 8 partials to every partition (stride-0 DRAM
            # read), reduce and reciprocal per partition, then scale
            zgb = spool.tile([P, NCORES], f32, name="zgb")
            zout_bc = bass.AP(
                zout.tensor, zout.offset, [(0, P), (1, NCORES)]
            )
            nc.sync.dma_start(out=zgb[:, :], in_=zout_bc)
            zp = spool.tile([P, 1], f32, name="zp")
            nc.vector.tensor_reduce(zp[:, 0:1], zgb[:, :], AX.X, ALU.add)
            rzb = spool.tile([P, 1], f32, name="rzb")
            nc.vector.reciprocal(rzb[:, 0:1], zp[:, 0:1])
            outsb = spool.tile([P, MT], f32, name="outsb")
            nc.vector.tensor_scalar_mul(outsb[:, :], expv[:, :], rzb[:, 0:1])
            nc.sync.dma_start(out=out_ext[:, :], in_=outsb[:, :])

    # run_bass_via_pjrt binds the exec primitive directly and skips the
    # finalize that bass_jit flows do; Bacc register allocation runs here.
    nc.finalize()
    _strip_redundant_ldweights(nc)
    return nc


def _strip_redundant_ldweights(nc):
    """Bacc's move_matmul_waits_to_ldweights emits one InstLdweights per
    matmul even when consecutive matmuls share the stationary operand.
    The PE keeps the loaded weights across matmuls, so an Ldweights whose
    weights AP equals the previous one's and that carries no semaphore
    waits/updates is pure redundant load time (~110ns each on the PE
    critical path). Drop them; only the matmuls (ldweights=false) remain."""
    def sig(arg):
        return (
            getattr(arg, "memref", None),
            getattr(arg, "offset", None),
            str(getattr(arg, "ap", None)),
        )

    removed = 0
    for bb in nc.main_func.blocks:
        keep = []
        last = None
        for inst in bb.instructions:
            if "Ldweights" in type(inst).__name__:
                s = sig(inst.ins[0])
                si = inst.sync_info
                if s == last and (
                    si is None or (not si.on_wait and not si.on_update)
                ):
                    removed += 1
                    continue
                last = s
            keep.append(inst)
        bb.instructions = keep
    return removed


def _prep_core_inputs(s, h, W, v):
    """Host-side layout prep: per-core tiled x^T, shared W^T, replicated v."""
    bf16 = ml_dtypes.bfloat16
    wt = np.ascontiguousarray(W.T).astype(bf16)          # [KIN, NOUT]
    vrep = np.ascontiguousarray(
        np.broadcast_to(v.reshape(1, NOUT), (P, NOUT))
    ).astype(np.float32)

    in_maps = []
    for c in range(NCORES):
        sl = slice(c * NS, (c + 1) * NS)
        x = np.concatenate([s[sl], h[sl]], axis=1)       # [NS, KIN]
        # xh[m*128+kk, k*128+rr] = x[m*128+rr, k*128+kk]
        xh = (
            x.reshape(MT, P, KT, P)
            .transpose(0, 3, 2, 1)
            .reshape(NS, KIN)
        )
        xh = np.ascontiguousarray(xh).astype(bf16)
        in_maps.append({"xh": xh, "wt": wt, "vr": vrep})
    return in_maps


_RUN_KW = {}  # test.py can inject trace=True etc.
LAST_RESULT = None


def kernel(s, h, W, v):
    from concourse.bass_utils import run_bass_kernel_spmd

    global LAST_RESULT
    s = np.asarray(s, dtype=np.float32)
    h = np.asarray(h, dtype=np.float32)
    W = np.asarray(W, dtype=np.float32)
    v = np.asarray(v, dtype=np.float32)

    in_maps = _prep_core_inputs(s, h, W, v)
    nc = _build_nc()
    res = run_bass_kernel_spmd(nc, in_maps, core_ids=list(range(NCORES)), **_RUN_KW)
    LAST_RESULT = res

    outs = []
    for c in range(NCORES):
        oc = np.asarray(res.results[c]["out"], dtype=np.float32)  # [P, MT]
        outs.append(oc.T.reshape(-1))                              # rows m*128+p
    return np.concatenate(outs).reshape(1, N).astype(np.float32)
